# revision 4
# baseline (speedup 1.0000x reference)
"""GAT (graph attention) kernel for Trainium2, 8-core SPMD.

Per core (dst-sharded), v2:
  Phase 1a: every core computes the full node table: row j = xw_bf16(128)
           (256B rows), xw = x @ W.  Stored in HBM in PARTITION-MAJOR
           layout (row order (p, tile) instead of (tile, p)) so the
           per-block table write is one contiguous ~1.5KB descriptor per
           partition instead of one 512B descriptor per node row.  The
           gather indices are permuted on host to match (free).
  Phase 1b: own-shard pass computes out_partial[d] = ee_loop[d]*xw[d]+bias
           directly into SBUF (bf16): psum = x_d@W + (1/ee_loop_d)*bias
           (rank-1 bias matmul), then ACT-engine copy with per-partition
           scale ee_loop.  This removes the separate self-loop table,
           its gather chunks, and the bias broadcast add in phase 2.
  Phase 2: edges partitioned by dst (host side), grouped into dst tiles
           of 128 and 128-edge chunks; groups of `gsz` tiles.  Per-edge
           source rows (256B) are fetched by dma_gather (4 src windows due
           to int16 gather indices, spread over 4 SWDGE queues); a
           one-hot-times-ee matrix M[e,d] (ee = host-normalized attention
           coef) routes each edge to its dst row; TensorE accumulates
           psum[128 dst, 128] += M^T @ G.  M is built on-device by the
           DVE (all-bf16 tensor_scalar: is_equal(iota, dmod) * ee) for
           most groups and uploaded from HBM for every `UPLOAD_MOD`-th
           group to balance DVE vs DMA.
           Final: out = relu(psum + out_partial), written p-major
           (host un-permutes).

Host precomputes per-edge normalized attention coefficients
(two matvecs + O(E) scalar math, ~0.2% of total FLOPs) and the edge->slot
schedule; the feature matmul, the edge gather, aggregation matmuls and the
nonlinearity run on device.  Softmax max-subtraction is unnecessary:
a_s, a_d ~ N(0,1) so logits stay O(10) and exp() is safe in fp32.
Padding slots get ee = 0 so they contribute nothing.
"""

import numpy as np
import ml_dtypes

BF16 = ml_dtypes.bfloat16

# problem constants (nn_GAT_43593918054566)
N_NODES = 100000
F_IN = 256
HID = 128
NEG_SLOPE = 0.2
N_CORES = 8
UPLOAD_NUM, UPLOAD_DEN = 4, 7   # upload M for NUM of every DEN groups


class Geo:
    """Geometry/schedule shared by host prep and kernel builder."""

    def __init__(self, n_nodes=N_NODES, f_in=F_IN, hid=HID, n_cores=N_CORES,
                 sh_tiles=98, group_tiles=5):
        self.n = n_nodes
        self.f_in = f_in
        self.hid = hid
        self.n_cores = n_cores
        self.ntiles_tab = -(-n_nodes // 128)          # node tiles in table
        self.ntab = self.ntiles_tab * 128             # padded table rows
        self.sh_tiles = sh_tiles                      # dst tiles per core
        self.sh = sh_tiles * 128                      # dst shard stride
        assert self.sh * (n_cores - 1) < n_nodes <= self.sh * n_cores
        # 4 src windows over PERMUTED rows (int16 gather index limit)
        nw = 4
        ws = -(-self.ntab // nw)
        assert ws <= 32768
        self.wb = [min(i * ws, self.ntab) for i in range(nw + 1)]
        self.gsz = group_tiles                        # dst tiles per group
        self.ng = -(-sh_tiles // group_tiles)

    def core_dst_range(self, c):
        lo = self.sh * c
        hi = min(lo + self.sh, self.n)
        return lo, hi

    def perm_row(self, n):
        """HBM table row of node n (partition-major layout)."""
        return (n & 127) * self.ntiles_tab + (n >> 7)


def _prep(geo, x, edge_index, W, att_src, att_dst, bias):
    """Host preprocessing: edge partitioning + per-core input arrays."""
    g = geo
    x = np.asarray(x, dtype=np.float32)
    W = np.asarray(W, dtype=np.float32)
    esrc = np.asarray(edge_index[0], dtype=np.int64)
    edst = np.asarray(edge_index[1], dtype=np.int64)

    # per-edge normalized attention (host: 2 matvecs + O(E) scalar math)
    a_s = x @ (W @ np.asarray(att_src, np.float32))
    a_d = x @ (W @ np.asarray(att_dst, np.float32))

    def ee_of(s, d):
        e = a_s[s] + a_d[d]
        e = np.where(e > 0, e, NEG_SLOPE * e)
        return np.exp(e).astype(np.float32)

    ee_reg_all = ee_of(esrc, edst)
    ee_loop = ee_of(np.arange(g.n), np.arange(g.n))   # self loops
    denom = ee_loop.astype(np.float64).copy()
    np.add.at(denom, edst, ee_reg_all.astype(np.float64))
    ee_reg_all = (ee_reg_all / denom[edst]).astype(np.float32)
    ee_loop = (ee_loop / denom).astype(np.float32)

    # permuted gather rows + window of every edge source
    rperm_all = g.perm_row(esrc)
    wbs = np.asarray(g.wb[1:], dtype=np.int64)

    cores = []
    for c in range(g.n_cores):
        lo, hi = g.core_dst_range(c)
        m = (edst >= lo) & (edst < hi)
        d_c = edst[m] - lo
        t_c = d_c >> 7
        rp_c = rperm_all[m]
        r_c = np.searchsorted(wbs, rp_c, side="right")
        cores.append((rp_c, d_c, t_c, r_c, ee_reg_all[m]))

    # regular-chunk quota per (tile, window): max over cores
    counts = np.zeros((g.n_cores, g.sh_tiles, 4), dtype=np.int64)
    for c, (_, _, t_c, r_c, _) in enumerate(cores):
        np.add.at(counts[c], (t_c, r_c), 1)
    C = -(-counts.max(axis=0) // 128)  # [T, 4]

    # chunk layout: per group, window-major regular chunks (no self chunks)
    chunk_off = np.zeros((g.sh_tiles, 4), dtype=np.int64)
    gather_segs = []   # (first_chunk, n_chunks, window)
    group_of = []      # (first_chunk, n_chunks, tiles)
    off = 0
    for gi in range(g.ng):
        tiles = list(range(gi * g.gsz, min((gi + 1) * g.gsz, g.sh_tiles)))
        g_first = off
        for r in range(4):
            seg_first = off
            for t in tiles:
                chunk_off[t, r] = off
                off += int(C[t, r])
            if off > seg_first:
                gather_segs.append((seg_first, off - seg_first, r))
        group_of.append((g_first, off - g_first, tiles))
    nch = off
    nslot = nch * 128

    # per-tile matmul chunk order: by window
    tile_chunks = {}
    for t in range(g.sh_tiles):
        ch = []
        for r in range(4):
            ch.extend(int(v) for v in
                      range(chunk_off[t, r], chunk_off[t, r] + C[t, r]))
        tile_chunks[t] = ch

    # which groups upload M from HBM (others are DVE-built on device)
    up_groups = [gi for gi in range(g.ng)
                 if (gi * UPLOAD_NUM) % UPLOAD_DEN < UPLOAD_NUM]
    mup_off = {}
    moff = 0
    for gi in up_groups:
        mup_off[gi] = moff
        moff += group_of[gi][1]
    nch_up = max(moff, 1)

    per_core = []
    for c, (rp_c, d_c, t_c, r_c, ee_c) in enumerate(cores):
        lo, hi = g.core_dst_range(c)
        idx_flat = np.zeros(nslot, dtype=np.int16)
        dmod = np.zeros(nslot, dtype=np.int32)
        eesl = np.zeros(nslot, dtype=np.float32)
        order = np.lexsort((r_c, t_c))
        rp_o, d_o, t_o, r_o = rp_c[order], d_c[order], t_c[order], r_c[order]
        ee_o = ee_c[order]
        run_id = t_o * 4 + r_o
        run_starts = np.searchsorted(run_id, np.arange(g.sh_tiles * 4))
        rank = np.arange(len(rp_o)) - run_starts[run_id]
        slot = chunk_off[t_o, r_o] * 128 + rank
        idx_flat[slot] = (rp_o - np.asarray(g.wb, dtype=np.int64)[r_o]).astype(np.int16)
        dmod[slot] = (d_o & 127).astype(np.int32)
        eesl[slot] = ee_o

        # wrap gather idx per segment: pos i -> [16k + i%16, i//16]
        idx16 = np.zeros((128, nslot // 16), dtype=np.int16)
        for seg_first, seg_nch, r in gather_segs:
            a, b = seg_first * 128, (seg_first + seg_nch) * 128
            wrapped = idx_flat[a:b].reshape(-1, 16).T
            cols = slice(a // 16, b // 16)
            for k in range(8):
                idx16[16 * k:16 * k + 16, cols] = wrapped
        dmodb = np.ascontiguousarray(
            dmod.reshape(nch, 128).T.astype(np.float32))
        eeb = np.ascontiguousarray(eesl.reshape(nch, 128).T)
        # host-built M for uploaded groups only
        mh = np.zeros((nch_up, 128, 128), dtype=BF16)
        for gi in up_groups:
            gfirst, gnch, _ = group_of[gi]
            mo = mup_off[gi]
            kk = np.arange(gnch * 128) // 128 + mo
            pp = np.arange(gnch * 128) % 128
            sl = slice(gfirst * 128, (gfirst + gnch) * 128)
            mh[kk, pp, dmod[sl]] = eesl[sl].astype(BF16)
        mh = np.ascontiguousarray(mh.transpose(1, 0, 2))  # [128, nch_up, 128]
        # own-shard x (transposed, zero-padded) + rank-1 bias row 1/ee_loop
        nd = hi - lo
        xto = np.zeros((g.f_in + 1, g.sh), dtype=BF16)
        xto[:g.f_in, :nd] = x[lo:hi].T.astype(BF16)
        xto[g.f_in, :nd] = (1.0 / ee_loop[lo:hi]).astype(BF16)
        eelp = np.zeros((128, g.sh_tiles), dtype=np.float32)
        eelp.reshape(-1)[:0] = 0  # noop, keep contiguous
        el = np.zeros(g.sh, dtype=np.float32)
        el[:nd] = ee_loop[lo:hi]
        eelp = np.ascontiguousarray(el.reshape(g.sh_tiles, 128).T)
        per_core.append({"idx": idx16, "mup": mh, "xto": xto,
                         "dmodb": dmodb, "eeb": eeb, "eeloop": eelp})

    xT = np.zeros((g.f_in, g.ntab), dtype=BF16)
    xT[:, :g.n] = x.T.astype(BF16)
    wbf = np.ascontiguousarray(W.astype(BF16))
    biasr = np.asarray(bias, np.float32).astype(BF16)[None, :]
    iota128 = np.ascontiguousarray(
        np.tile(np.arange(128, dtype=np.float32).astype(BF16), (128, 1)))

    shared = {"xt": xT, "w": wbf, "biasr": biasr, "iota128": iota128}
    sched = {"C": C, "nch": nch, "nslot": nslot, "nch_up": nch_up,
             "gather_segs": gather_segs, "group_of": group_of,
             "tile_chunks": tile_chunks, "up_groups": set(up_groups),
             "mup_off": mup_off}
    return shared, per_core, sched


def _build(geo, sched):
    """Build the (core-uniform) Bass program."""
    import concourse.bacc as bacc
    import concourse.mybir as mybir
    from concourse import tile
    from contextlib import ExitStack

    g = geo
    nch, nslot = sched["nch"], sched["nslot"]
    f32, bf16 = mybir.dt.float32, mybir.dt.bfloat16
    i16 = mybir.dt.int16
    Alu = mybir.AluOpType

    nc = bacc.Bacc("TRN2", target_bir_lowering=False, debug=False,
                   num_devices=g.n_cores, num_swdge_queues=4)

    xt_d = nc.dram_tensor("xt", [g.f_in, g.ntab], bf16, kind="ExternalInput")
    xto_d = nc.dram_tensor("xto", [g.f_in + 1, g.sh], bf16, kind="ExternalInput")
    w_d = nc.dram_tensor("w", [g.f_in, g.hid], bf16, kind="ExternalInput")
    biasr_d = nc.dram_tensor("biasr", [1, g.hid], bf16, kind="ExternalInput")
    idx_d = nc.dram_tensor("idx", [128, nslot // 16], i16, kind="ExternalInput")
    mup_d = nc.dram_tensor("mup", [128, sched["nch_up"], 128], bf16,
                           kind="ExternalInput")
    iota_d = nc.dram_tensor("iota128", [128, 128], bf16, kind="ExternalInput")
    dmodb_d = nc.dram_tensor("dmodb", [128, nch], f32, kind="ExternalInput")
    eeb_d = nc.dram_tensor("eeb", [128, nch], f32, kind="ExternalInput")
    eeloop_d = nc.dram_tensor("eeloop", [128, g.sh_tiles], f32,
                              kind="ExternalInput")
    out_d = nc.dram_tensor("out", [128, g.sh_tiles, g.hid], f32,
                           kind="ExternalOutput")
    table_d = nc.dram_tensor("table", [g.ntab, 128], bf16, kind="Internal")
    table_pm = table_d.rearrange("(p a) e -> p a e", p=128)

    with tile.TileContext(nc) as tc, ExitStack() as ctx:
        const = ctx.enter_context(tc.tile_pool(name="const", bufs=1))
        w0 = const.tile([128, g.hid], bf16)
        w1 = const.tile([128, g.hid], bf16)
        nc.sync.dma_start(w0[:], w_d[0:128, :])
        nc.sync.dma_start(w1[:], w_d[128:256, :])
        biasr_sb = const.tile([1, g.hid], bf16)
        nc.sync.dma_start(biasr_sb[:], biasr_d[:])
        idx_sb = const.tile([128, nslot // 16], i16)
        nc.sync.dma_start(idx_sb[:], idx_d[:])
        iota_sb = const.tile([128, 128], bf16)
        nc.sync.dma_start(iota_sb[:], iota_d[:])
        dmodb_sb = const.tile([128, nch], f32)
        nc.sync.dma_start(dmodb_sb[:], dmodb_d[:])
        eeb_sb = const.tile([128, nch], f32)
        nc.sync.dma_start(eeb_sb[:], eeb_d[:])
        eeloop_sb = const.tile([128, g.sh_tiles], f32)
        nc.sync.dma_start(eeloop_sb[:], eeloop_d[:])
        outp = const.tile([128, g.sh_tiles, g.hid], bf16)

        # ---- Phase 1a: global node table (p-major 256B rows) ----
        with tc.tile_pool(name="xp", bufs=4) as xp, \
             tc.tile_pool(name="stag", bufs=3) as stag, \
             tc.tile_pool(name="ps1", bufs=7, space="PSUM") as ps1:
            for b in range(-(-g.ntiles_tab // 6)):
                t0 = 6 * b
                nt = min(6, g.ntiles_tab - t0)
                xs0 = xp.tile([128, nt * 128], bf16, tag="xs0")
                xs1 = xp.tile([128, nt * 128], bf16, tag="xs1")
                nc.sync.dma_start(xs0[:], xt_d[0:128, t0 * 128:(t0 + nt) * 128])
                nc.sync.dma_start(xs1[:], xt_d[128:256, t0 * 128:(t0 + nt) * 128])
                s = stag.tile([128, nt, 128], bf16, tag="s")
                for h in range(-(-nt // 3)):
                    np_ = min(3, nt - 3 * h)
                    ps = ps1.tile([128, np_ * 128], f32, tag="ps1t")
                    for j in range(np_):
                        jj = 3 * h + j
                        nc.tensor.matmul(ps[:, j * 128:(j + 1) * 128],
                                         xs0[:, jj * 128:(jj + 1) * 128],
                                         w0[:], start=True, stop=False)
                        nc.tensor.matmul(ps[:, j * 128:(j + 1) * 128],
                                         xs1[:, jj * 128:(jj + 1) * 128],
                                         w1[:], start=False, stop=True)
                    nc.scalar.copy(s[:, 3 * h:3 * h + np_, :],
                                   ps[:].rearrange("p (a b) -> p a b", b=128))
                nc.scalar.dma_start(table_pm[:, t0:t0 + nt, :], s[:])

            # ---- Phase 1b: own shard -> out_partial ----
            for b in range(-(-g.sh_tiles // 6)):
                t0 = 6 * b
                nt = min(6, g.sh_tiles - t0)
                xs0 = xp.tile([128, nt * 128], bf16, tag="xs0")
                xs1 = xp.tile([128, nt * 128], bf16, tag="xs1")
                xs2 = xp.tile([1, nt * 128], bf16, tag="xs2")
                nc.sync.dma_start(xs0[:], xto_d[0:128, t0 * 128:(t0 + nt) * 128])
                nc.sync.dma_start(xs1[:], xto_d[128:256, t0 * 128:(t0 + nt) * 128])
                nc.sync.dma_start(xs2[:], xto_d[256:257, t0 * 128:(t0 + nt) * 128])
                for h in range(-(-nt // 3)):
                    np_ = min(3, nt - 3 * h)
                    ps = ps1.tile([128, np_ * 128], f32, tag="ps1t")
                    for j in range(np_):
                        jj = 3 * h + j
                        nc.tensor.matmul(ps[:, j * 128:(j + 1) * 128],
                                         xs0[:, jj * 128:(jj + 1) * 128],
                                         w0[:], start=True, stop=False)
                        nc.tensor.matmul(ps[:, j * 128:(j + 1) * 128],
                                         xs1[:, jj * 128:(jj + 1) * 128],
                                         w1[:], start=False, stop=False)
                        nc.tensor.matmul(ps[:, j * 128:(j + 1) * 128],
                                         xs2[:, jj * 128:(jj + 1) * 128],
                                         biasr_sb[:], start=False, stop=True)
                    for j in range(np_):
                        t = t0 + 3 * h + j
                        nc.scalar.mul(outp[:, t, :],
                                      ps[:, j * 128:(j + 1) * 128],
                                      eeloop_sb[:, t:t + 1])

        # ---- Phase 2: gather + attention aggregation ----
        with tc.tile_pool(name="gp", bufs=3) as gp, \
             tc.tile_pool(name="mp", bufs=2) as mp, \
             tc.tile_pool(name="ps2", bufs=8, space="PSUM") as ps2, \
             tc.tile_pool(name="op", bufs=3) as op:
            segs_by_group = {}
            for seg_first, seg_nch, r in sched["gather_segs"]:
                for gi, (gfirst, gnch, tiles) in enumerate(sched["group_of"]):
                    if gfirst <= seg_first < gfirst + gnch:
                        segs_by_group.setdefault(gi, []).append(
                            (seg_first, seg_nch, r))
                        break
            tile_chunks = sched["tile_chunks"]
            qn = 0
            for gi, (gfirst, gnch, tiles) in enumerate(sched["group_of"]):
                G = gp.tile([128, gnch, 128], bf16, tag="G")
                for seg_first, seg_nch, r in segs_by_group.get(gi, []):
                    lo = seg_first - gfirst
                    nc.gpsimd.dma_gather(
                        G[:, lo:lo + seg_nch, :],
                        table_d[g.wb[r]:g.wb[r + 1], :],
                        idx_sb[:, seg_first * 8:(seg_first + seg_nch) * 8],
                        seg_nch * 128, seg_nch * 128, 128,
                        single_packet=False, queue_num=qn % 4)
                    qn += 1
                M = mp.tile([128, gnch, 128], bf16, tag="M")
                if gi in sched["up_groups"]:
                    mo = sched["mup_off"][gi]
                    nc.sync.dma_start(M[:], mup_d[:, mo:mo + gnch, :])
                else:
                    for k in range(gnch):
                        ka = gfirst + k
                        nc.vector.tensor_scalar(
                            M[:, k, :], iota_sb[:],
                            dmodb_sb[:, ka:ka + 1], eeb_sb[:, ka:ka + 1],
                            Alu.is_equal, Alu.mult)
                obg = op.tile([128, len(tiles), g.hid], f32, tag="obg")
                for ti, t in enumerate(tiles):
                    ch = tile_chunks[t]
                    pst = ps2.tile([128, g.hid], f32, tag="pst",
                                   name=f"pst{t}")
                    for i, k in enumerate(ch):
                        nc.tensor.matmul(pst[:],
                                         M[:, k - gfirst, :],
                                         G[:, k - gfirst, :],
                                         start=(i == 0), stop=(i == len(ch) - 1))
                    nc.vector.tensor_tensor(obg[:, ti, :], pst[:],
                                            outp[:, t, :], Alu.add)
                    nc.vector.tensor_scalar(obg[:, ti, :], obg[:, ti, :], 0.0,
                                            None, Alu.max)
                nc.scalar.dma_start(
                    out_d[:, tiles[0]:tiles[0] + len(tiles), :],
                    obg[:, 0:len(tiles), :])
    nc.compile()
    return nc


def _in_maps(geo, shared, per_core):
    maps = []
    for c in range(geo.n_cores):
        m = dict(shared)
        m.update(per_core[c])
        maps.append(m)
    return maps


def _unshard(geo, res):
    """Assemble the full [N, HID] output from per-core p-major outputs."""
    outs = []
    for c in range(geo.n_cores):
        lo, hi = geo.core_dst_range(c)
        o = res.results[c]["out"]                      # [128, sh_tiles, hid]
        o = np.ascontiguousarray(o.transpose(1, 0, 2)).reshape(geo.sh, geo.hid)
        outs.append(o[:hi - lo])
    return np.concatenate(outs, axis=0).astype(np.float32)


def kernel(x, edge_index, W, att_src, att_dst, bias):
    from concourse.bass_utils import run_bass_kernel_spmd

    geo = Geo()
    shared, per_core, sched = _prep(geo, x, edge_index, W, att_src, att_dst, bias)
    nc = _build(geo, sched)
    in_maps = _in_maps(geo, shared, per_core)
    res = run_bass_kernel_spmd(nc, in_maps, core_ids=list(range(geo.n_cores)))
    return _unshard(geo, res)


def _emulate(geo, shared, per_core, sched):
    """Numpy emulation of the device program (for host-side validation)."""
    g = geo
    xT = shared["xt"].astype(np.float32)
    w = shared["w"].astype(np.float32)
    biasr = shared["biasr"].astype(np.float32)
    table = (xT.T @ w).astype(BF16)                    # [ntab, hid] bf16
    # permuted table: row perm_row(n) = node n
    ptab = np.zeros_like(table)
    nids = np.arange(g.ntab)
    ptab[(nids & 127) * g.ntiles_tab + (nids >> 7)] = table
    outs = []
    for c in range(g.n_cores):
        pc = per_core[c]
        lo, hi = g.core_dst_range(c)
        xto = pc["xto"].astype(np.float32)
        psum_own = (xto[:g.f_in].T @ w) + np.outer(xto[g.f_in], biasr[0])
        eelp = pc["eeloop"]                            # [128, sh_tiles]
        outp = np.zeros((g.sh, g.hid), dtype=np.float32)
        for t in range(g.sh_tiles):
            sl = slice(t * 128, (t + 1) * 128)
            outp[sl] = psum_own[sl] * eelp[:, t][:, None]
        outp = outp.astype(BF16).astype(np.float32)
        # gather + aggregate
        idx16 = pc["idx"]
        nch = sched["nch"]
        G = np.zeros((nch * 128, g.hid), dtype=np.float32)
        for seg_first, seg_nch, r in sched["gather_segs"]:
            a, b = seg_first * 128, (seg_first + seg_nch) * 128
            flat = idx16[0:16, a // 16:b // 16].T.reshape(-1)
            G[a:b] = ptab[g.wb[r] + flat.astype(np.int64)]
        dmodb = pc["dmodb"].astype(np.int32)           # [128, nch]
        eeb = pc["eeb"].astype(np.float32)
        out = np.zeros((g.sh, g.hid), dtype=np.float32)
        for t in range(g.sh_tiles):
            acc = np.zeros((128, g.hid), dtype=np.float32)
            for k in sched["tile_chunks"][t]:
                Gk = G[k * 128:(k + 1) * 128]
                M = np.zeros((128, 128), dtype=np.float32)
                M[np.arange(128), dmodb[:, k]] = eeb[:, k]
                acc += (M.astype(BF16).astype(np.float32).T
                        @ Gk.astype(BF16).astype(np.float32))
            out[t * 128:(t + 1) * 128] = np.maximum(
                acc + outp[t * 128:(t + 1) * 128], 0.0)
        outs.append(out[:hi - lo])
    return np.concatenate(outs, axis=0)


if __name__ == "__main__":
    rng = np.random.RandomState(0)
    geo = Geo(n_nodes=2048, sh_tiles=2, group_tiles=2)
    x = rng.randn(2048, 256).astype(np.float32)
    ei = rng.randint(0, 2048, (2, 8192)).astype(np.int64)
    W = rng.randn(256, 128).astype(np.float32) / 16
    a1 = rng.randn(128).astype(np.float32) / 11.3
    a2 = rng.randn(128).astype(np.float32) / 11.3
    b = np.zeros(128, np.float32)
    sh, pc, sc = _prep(geo, x, ei, W, a1, a2, b)
    print("nch:", sc["nch"], "nslot:", sc["nslot"], "nch_up:", sc["nch_up"])
    # emulate vs reference
    import jax, jax.numpy as jnp
    def ref(x, ei, W, a1, a2, b):
        N = x.shape[0]
        xw = x @ W
        loops = np.arange(N)
        src = np.concatenate([ei[0], loops])
        dst = np.concatenate([ei[1], loops])
        a_s = xw @ a1; a_d = xw @ a2
        e = a_s[src] + a_d[dst]
        e = np.where(e > 0, e, 0.2 * e)
        ee = np.exp(e)
        denom = np.zeros(N); np.add.at(denom, dst, ee)
        coef = ee / denom[dst]
        out = np.zeros((N, 128))
        np.add.at(out, dst, xw[src] * coef[:, None])
        return np.maximum(out + b, 0)
    exp = ref(x, ei, W, a1, a2, b)
    act = _emulate(geo, sh, pc, sc)
    rel = np.linalg.norm(act - exp) / np.linalg.norm(exp)
    print("emulation rel err:", rel)
